# revision 45
# baseline (speedup 1.0000x reference)
"""Trainium2 Bass kernel for a gated bilinear-attention GNN (GAT-with-gate).

Math (per batch b):
    h   = x @ W_w.T + W_b                      [N, D]
    e   = h (A + A^T) h^T  (symmetric)         [N, N]
    E   = exp(e + maskbias - s)                (masked attention, T layout)
    den = rowsum(E) + (N - indeg) * exp(-s)
    rv  = h; 3x: az = relu(att @ rv) with att = E/den;
               c = sigmoid([h, az] @ gate_w.T + gate_b)
               rv = c * h + (1 - c) * az

Data-parallel over batch: 2 batches/core on 8 cores; full inputs sharded on
the host, outputs gathered.  Per core, the two batches' phases are manually
interleaved so every engine sees cross-batch work (the PE dispatches its
FIFO in order, so neighbor-phase matmuls are woven between dependency
stalls).

Key design points vs the v1 kernel (97.2us -> ~78-80us):
  - The adjacency mask is applied INSIDE the logit PSUM accumulation: the
    host ships adjbias in {0, -60} bf16 and one extra matmul per slab
    (identity.T @ adjbias) adds it onto e.  The ACT exp then emits masked
    attention directly with fused row sums - the entire N^2 DVE mask pass
    (~19us) and any attE intermediate disappear.
  - attT and the hop stationaries are fp8e4 (logits shifted by s=7, sv
    scaled by 32 against denormals); the hop matmuls run in DoubleRow mode
    (2 contraction rows/cycle), each stationary load shared by both moving
    halves.  rel err ~3.5e-3 vs the 2e-2 gate.
  - gate pre-activations z2 are per-128-node-block matmuls (stationary =
    az block, moving = both gate vectors) landing in [128, 2*NB] PSUM; no
    [1, N] single-partition ops anywhere.  z1 (h part) is computed once.
  - combine is rv = bv*(az - h) + sv0: one wide scalar-free subtract plus
    one per-block scalar_tensor_tensor, reusing the hop-0 stationary sv0
    as the precomputed inv-scaled h term.
  - PSUM: e-slabs [128,1024]x2, az [128,512]x2 (stationary-shared ih
    pairs), transposes [128,512]x1 bf16, gate [128,16]x1.
  - all large evictions are >=512 wide; ACT handles relu/exp, DVE the
    tensor-tensor work; PE warmup matmuls run off memset tiles before the
    first DMA lands.
"""

import sys
from contextlib import ExitStack

import ml_dtypes
import numpy as np

sys.path.insert(0, "/opt/trn_rl_repo")

import concourse.bass as bass
import concourse.tile as tile
from concourse import mybir
from concourse.bass_utils import run_bass_kernel_spmd


B, N, D = 16, 1024, 128
NCORES = 8
BPC = B // NCORES        # batches per core
NB = N // 128            # 128-wide blocks per matrix dim
F32 = mybir.dt.float32
F32R = mybir.dt.float32r
BF16 = mybir.dt.bfloat16
OP = mybir.AluOpType
AF = mybir.ActivationFunctionType

S_SHIFT = 7.0            # logit shift: exp(e-s) <= ~200 fits fp8e4 range
CBIG = 60.0              # mask bias: non-edges get exp(e - CBIG) ~ 0
KSC = 32.0               # sv scale: sv = rv*inv*KSC keeps fp8 out of denormals
FP8 = mybir.dt.float8e4

# consts blob column offsets (f32 [128, CB_W])
CB_WWT = 0        # [128, 128] W_w.T  (hT = WwT.T @ xT)
CB_A = 128        # [128, 128] A
CB_ID = 256       # [128, 128] identity f32
CB_WB = 384       # [128, 1]   W_b per-partition
CB_GW = 385       # [128, 2]   gate_w as two columns (h part, az part)
CB_NGB = 387      # [128, 1]   -gate_b
CB_NS = 388       # [128, 1]   -S_SHIFT
CB_W = 389


def build_nc():
    nc = bass.Bass("TRN2", target_bir_lowering=False, debug=False,
                   num_devices=NCORES)

    adjT = nc.dram_tensor("adjT", [BPC, 128, NB * N], FP8,
                          kind="ExternalInput").ap()
    xnb = nc.dram_tensor("xnb", [BPC, 128, N + NB], F32,
                         kind="ExternalInput").ap()
    cblob = nc.dram_tensor("cblob", [128, CB_W], F32,
                           kind="ExternalInput").ap()
    out = nc.dram_tensor("out", [BPC, NB, 128, D], F32,
                         kind="ExternalOutput").ap()

    with tile.TileContext(nc) as tc, ExitStack() as ctx:
        consts = ctx.enter_context(tc.tile_pool(name="consts", bufs=1))
        ps_e = ctx.enter_context(tc.tile_pool(name="ps_e", bufs=2,
                                              space="PSUM"))
        ps_az = ctx.enter_context(tc.tile_pool(name="ps_az", bufs=2,
                                               space="PSUM"))
        ps_trb = ctx.enter_context(tc.tile_pool(name="ps_trb", bufs=1,
                                                space="PSUM"))
        ps_z = ctx.enter_context(tc.tile_pool(name="ps_z", bufs=1,
                                              space="PSUM"))
        big = ctx.enter_context(tc.tile_pool(name="big", bufs=2))
        work = ctx.enter_context(tc.tile_pool(name="work", bufs=2))
        hopp = ctx.enter_context(tc.tile_pool(name="hopp", bufs=4))
        tiny = ctx.enter_context(tc.tile_pool(name="tiny", bufs=8))

        cb = consts.tile([128, CB_W], F32, tag="cb")
        nc.sync.dma_start(cb[:, :], cblob[:, :])
        wwT = cb[:, CB_WWT:CB_WWT + 128]
        ident = cb[:, CB_ID:CB_ID + 128]
        wb = cb[:, CB_WB:CB_WB + 1]
        gwc = cb[:, CB_GW:CB_GW + 2]
        ngb = cb[:, CB_NGB:CB_NGB + 1]
        nsh = cb[:, CB_NS:CB_NS + 1]

        identr = consts.tile([128, 128], F32R, tag="identr")
        nc.vector.tensor_copy(identr[:, :], ident[:, :])
        identb = consts.tile([128, 128], BF16, tag="identb")
        nc.vector.tensor_copy(identb[:, :], ident[:, :])
        identf = consts.tile([128, 128], FP8, tag="identf")
        nc.vector.tensor_copy(identf[:, :], ident[:, :])
        gwb = consts.tile([128, 2], BF16, tag="gwb")
        nc.vector.tensor_copy(gwb[:, :], gwc[:, :])
        gwr = consts.tile([128, 2], F32R, tag="gwr")
        nc.vector.tensor_copy(gwr[:, :], gwc[:, :])

        # PE warmup: real matmuls fed from memset tiles (no DMA dep) so
        # the HAM clock-gate opens before the first real matmuls issue.
        wsrc = consts.tile([128, 512], F32, tag="wsrc")
        nc.vector.memset(wsrc[:, :], 0.125)
        warm_ps = ps_az.tile([128, 512], F32, tag="ps_az")
        for _ in range(4):
            nc.tensor.matmul(warm_ps[0:8, :], wsrc[:, 0:8],
                             wsrc[:, :], start=True, stop=True)

        # ACT exp table preload (hidden under startup DMAs)
        tdum = consts.tile([1, 1], F32, tag="tdum")
        nc.scalar.activation(tdum[:, :], wsrc[0:1, 0:1], AF.Exp)

        # S = A + A^T (f32r, stays for the whole kernel)
        s_sb = consts.tile([D, D], F32R, tag="smat")
        at_ps = ps_az.tile([128, 512], F32, tag="ps_az")
        nc.tensor.transpose(at_ps[:, 0:128], cb[:, CB_A:CB_A + 128],
                            ident[:, :])
        nc.vector.tensor_tensor(s_sb[:, :], cb[:, CB_A:CB_A + 128],
                                at_ps[:, 0:128], OP.add)

        def phase_setup(b, st):
            """h (T + natural layouts), hS, gate z1; start adj DMA."""
            xn = work.tile([128, N + NB], F32, tag="xn")
            nc.sync.dma_start(xn[:, :], xnb[b, :, :])
            adj_sb = big.tile([128, NB * N], FP8, tag="adj")
            for hh in range(2):
                nc.sync.dma_start(adj_sb[:, hh * 4096:(hh + 1) * 4096],
                                  adjT[b, :, hh * 4096:(hh + 1) * 4096])

            # hT = WwT.T @ xT + Wb  (plain f32: rhs comes straight from DMA)
            hT = work.tile([128, N], F32R, tag="hT")
            for ih in range(2):
                ph = ps_az.tile([128, 512], F32, tag="ps_az", name="ph")
                nc.tensor.matmul(ph[:, :], wwT,
                                 xn[:, ih * 512:(ih + 1) * 512],
                                 start=True, stop=True)
                nc.vector.tensor_scalar(hT[:, ih * 512:(ih + 1) * 512],
                                         ph[:, :], wb, None, OP.add)

            # hST = S @ hT
            hST = work.tile([128, N], F32R, tag="hST")
            for ih in range(2):
                ph2 = ps_az.tile([128, 512], F32, tag="ps_az", name="ph2")
                nc.tensor.matmul(ph2[:, :], s_sb,
                                 hT[:, ih * 512:(ih + 1) * 512],
                                 start=True, stop=True)
                nc.vector.tensor_copy(hST[:, ih * 512:(ih + 1) * 512],
                                      ph2[:, :])

            # hnat[p, nb*128+f] = h[nb*128+p, f]  (transposes land in the
            # az psum pool, which is idle during setup)
            hnat = work.tile([128, N], F32, tag="hnat")
            for hh in range(2):
                ptn = ps_az.tile([128, 512], F32, tag="ps_az", name="ptn")
                for q in range(4):
                    nb = hh * 4 + q
                    nc.tensor.transpose(
                        ptn[:, q * 128:(q + 1) * 128].bitcast(F32R),
                        hT[:, nb * 128:(nb + 1) * 128], identr)
                nc.vector.tensor_copy(hnat[:, hh * 512:(hh + 1) * 512],
                                      ptn[:, :])

            # z1[p, nb] = sum_f gw1[f] h[nb*128+p, f]  (2-col moving: PSUM
            # writes need >=8B; col 1 is discarded)
            pz1 = ps_z.tile([128, 2 * NB], F32, tag="ps_z")
            for q in range(NB):
                nc.tensor.matmul(pz1[:, 2 * q:2 * q + 2],
                                 hT[:, q * 128:(q + 1) * 128],
                                 gwr[:, 0:2], start=True, stop=True)
            z1 = tiny.tile([128, NB], F32, tag="z1")
            nc.vector.tensor_copy(z1[:, :], pz1[:, 0:2 * NB:2])
            st.update(hT=hT, hST=hST, hnat=hnat, z1=z1, adj=adj_sb,
                      xnhandle=xn)

        def att_slab(b, st, jb):
            """One jb slab: e-matmuls + mask accumulation + exp."""
            hT, hST, adj_sb = st["hT"], st["hST"], st["adj"]
            if jb == 0:
                st["att"] = big.tile([128, NB * N], FP8, tag="attT",
                                     name="attT")
                st["acc2"] = tiny.tile([128, 2 * NB], F32, tag="acc2",
                                       name="acc2")
            attT, acc2 = st["att"], st["acc2"]
            pe = ps_e.tile([128, N], F32, tag="ps_e", name="pe")
            for ih in range(2):
                sl = slice(ih * 512, (ih + 1) * 512)
                nc.tensor.matmul(pe[:, sl],
                                 hST[:, jb * 128:(jb + 1) * 128],
                                 hT[:, sl], start=True, stop=False)
                nc.tensor.matmul(pe[:, sl], identf,
                                 adj_sb[:, jb * N + ih * 512:
                                        jb * N + (ih + 1) * 512],
                                 start=False, stop=True)
            nc.scalar.activation(attT[:, jb * N:(jb + 1) * N], pe[:, :],
                                 AF.Exp, bias=nsh,
                                 accum_out=acc2[:, 2 * jb:2 * jb + 1])

        def att_fin(b, st):
            """denominator -> invK; hop-1 stationary sv0."""
            acc2 = st["acc2"]
            inv = tiny.tile([128, NB], F32, tag="inv")
            nc.vector.tensor_tensor(inv[:, :], acc2[:, 0:2 * NB:2],
                                    st["xnhandle"][:, N:N + NB], OP.add)
            nc.vector.reciprocal(inv[:, :], inv[:, :])
            invK = tiny.tile([128, NB], F32, tag="invK")
            nc.vector.tensor_scalar(invK[:, :], inv[:, :], KSC, None, OP.mult)
            sv0 = hopp.tile([128, N], FP8, tag="sv0", bufs=2, name="sv0")
            hnat = st["hnat"]
            for q in range(NB):
                nc.vector.tensor_scalar(sv0[:, q * 128:(q + 1) * 128],
                                        hnat[:, q * 128:(q + 1) * 128],
                                        invK[:, q:q + 1], None, OP.mult)
            st.update(invK=invK, sv=sv0, sv0=sv0)

        def hop_mm(b, st, k):
            """az matmuls (DoubleRow fp8), stationary shared across both
            ih halves so every LDWEIGHTS feeds two matmuls."""
            attT, sv = st["att"], st["sv"]
            attT3 = attT.rearrange("p (jb i) -> p jb i", jb=NB)
            sv3 = sv.rearrange("p (nb f) -> p nb f", nb=NB)
            azT = hopp.tile([128, N], BF16, tag="azT", name="azT")
            paz0 = ps_az.tile([128, 512], F32, tag="ps_az", name="paz0")
            paz1 = ps_az.tile([128, 512], F32, tag="ps_az", name="paz1")
            pz = (paz0, paz1)
            for jp in range(NB // 2):
                for ih in range(2):
                    nc.tensor.matmul(
                        pz[ih][:, :],
                        sv3[:, 2 * jp:2 * jp + 2, :],
                        attT3[:, 2 * jp:2 * jp + 2,
                              ih * 512:(ih + 1) * 512],
                        start=(jp == 0), stop=(jp == NB // 2 - 1),
                        perf_mode=mybir.MatmulPerfMode.DoubleRow)
            for ih in range(2):
                nc.scalar.activation(azT[:, ih * 512:(ih + 1) * 512],
                                     pz[ih][:, :], AF.Relu, scale=1.0 / KSC)
            st["azT"] = azT

        def hop_mid(b, st, k, ih):
            """One half: az transposes + gate z2 matmuls + d = az - h."""
            azT, hnat = st["azT"], st["hnat"]
            if ih == 0:
                st["pz2"] = ps_z.tile([128, 2 * NB], F32, tag="ps_z",
                                      name="pz2")
                st["dd"] = hopp.tile([128, N], F32, tag="dd", name="dd")
            pz2, dd = st["pz2"], st["dd"]
            ptz = ps_trb.tile([128, 512], BF16, tag="ps_trb", name="ptz")
            for q in range(4):
                nb = ih * 4 + q
                nc.tensor.transpose(ptz[:, q * 128:(q + 1) * 128],
                                    azT[:, nb * 128:(nb + 1) * 128],
                                    identb)
                nc.tensor.matmul(pz2[:, 2 * nb:2 * nb + 2],
                                 azT[:, nb * 128:(nb + 1) * 128],
                                 gwb[:, 0:2], start=True, stop=True)
            nc.vector.tensor_tensor(dd[:, ih * 512:(ih + 1) * 512],
                                    ptz[:, :],
                                    hnat[:, ih * 512:(ih + 1) * 512],
                                    OP.subtract)

        def hop_tail(b, st, k, ih):
            """One half: gate sigmoid weights + combine + (last) store."""
            last = (k == 2)
            invK, sv0 = st["invK"], st["sv0"]
            hnat, pz2, dd = st["hnat"], st["pz2"], st["dd"]
            z1 = st["z1"]
            hs = slice(ih * 4, ih * 4 + 4)
            # E = exp(-(z1+z2)-gb); w2 = 1-sigmoid = E/(1+E)
            zt = tiny.tile([128, 4], F32, tag="zt", name="zt")
            nc.vector.tensor_tensor(zt[:, :],
                                    pz2[:, 8 * ih + 1:8 * ih + 8:2],
                                    z1[:, hs], OP.add)
            egt = tiny.tile([128, 4], F32, tag="egt", name="egt")
            nc.scalar.activation(egt[:, :], zt[:, :], AF.Exp, bias=ngb,
                                 scale=-1.0)
            u8 = tiny.tile([128, 4], F32, tag="u8", name="u8")
            nc.vector.tensor_scalar(u8[:, :], egt[:, :], 1.0, None, OP.add)
            nc.vector.reciprocal(u8[:, :], u8[:, :])
            bv = tiny.tile([128, 4], F32, tag="bv", name="bv")
            if last:
                nc.vector.tensor_tensor(bv[:, :], egt[:, :], u8[:, :],
                                        OP.mult)
            else:
                t8 = tiny.tile([128, 4], F32, tag="t8", name="t8")
                nc.vector.tensor_tensor(t8[:, :], egt[:, :], invK[:, hs],
                                        OP.mult)
                nc.vector.tensor_tensor(bv[:, :], t8[:, :], u8[:, :],
                                        OP.mult)

            # combine: sv_next = bv*(az-h) + invK*h = bv*d + sv0
            #          (last)   rv = (1-c)*d + h
            if ih == 0:
                if last:
                    st["rv"] = hopp.tile([128, N], F32, tag="rvout",
                                         name="rvout")
                else:
                    st["rv"] = hopp.tile([128, N], FP8, tag="sv",
                                         name="svnext")
            rv = st["rv"]
            base = hnat if last else sv0
            for q in range(4):
                nb = ih * 4 + q
                sl = slice(nb * 128, (nb + 1) * 128)
                nc.vector.scalar_tensor_tensor(rv[:, sl], dd[:, sl],
                                               bv[:, q:q + 1], base[:, sl],
                                               OP.mult, OP.add)
            if last:
                nc.sync.dma_start(
                    out[b, ih * 4:(ih + 1) * 4].transpose([1, 0, 2]),
                    rv[:, ih * 512:(ih + 1) * 512]
                    .rearrange("p (nb d) -> p nb d", nb=4))
            if not last and ih == 1:
                st["sv"] = rv

        # att0 slabs interleaved with setup1 so the PE FIFO never blocks;
        # hops paired per k across batches.
        s0, s1 = {}, {}
        phase_setup(0, s0)
        for jb in range(3):
            att_slab(0, s0, jb)
        phase_setup(1, s1)
        for jb in range(3, NB):
            att_slab(0, s0, jb)
        att_fin(0, s0)
        for jb in range(2):
            att_slab(1, s1, jb)
        hop_mm(0, s0, 0)
        for jb in range(2, 5):
            att_slab(1, s1, jb)
        hop_mid(0, s0, 0, 0)
        hop_mid(0, s0, 0, 1)
        for jb in range(5, NB):
            att_slab(1, s1, jb)
        att_fin(1, s1)
        hop_tail(0, s0, 0, 0)
        hop_tail(0, s0, 0, 1)
        hop_mm(1, s1, 0)
        hop_mid(1, s1, 0, 0)
        hop_mid(1, s1, 0, 1)
        hop_tail(1, s1, 0, 0)
        hop_tail(1, s1, 0, 1)
        for k in range(1, 3):
            hop_mm(0, s0, k)
            hop_mm(1, s1, k)
            hop_mid(0, s0, k, 0)
            hop_mid(0, s0, k, 1)
            hop_mid(1, s1, k, 0)
            hop_tail(0, s0, k, 0)
            hop_mid(1, s1, k, 1)
            hop_tail(0, s0, k, 1)
            hop_tail(1, s1, k, 0)
            hop_tail(1, s1, k, 1)

        nop_insts = []
        for eng in (nc.tensor, nc.vector, nc.scalar, nc.gpsimd, nc.sync):
            for _ in range(128):
                nop_insts.append(eng.nop(nofuse=True).ins)

    _fixup_waits(nc, nop_insts)
    return nc


_FIXUP_SKIP = {"InstNoOp"}


def _fixup_waits(nc, nop_insts):
    """walrus (enable-ldw-opt=false) rejects compute instructions with more
    than one sync wait (single wait slot in the S3 structs).  Hoist
    all-but-one wait of each such instruction onto spare same-engine nop
    instructions inserted immediately before it in program order."""
    nop_set = set(id(x) for x in nop_insts)
    free_nops = {}
    for x in nop_insts:
        free_nops.setdefault(x.engine, []).append(x)
    f = nc.m.functions[0]
    for blk in f.blocks:
        insts = blk.instructions
        for i in range(len(insts) - 1, -1, -1):
            if id(insts[i]) in nop_set:
                insts.pop(i)
        i = 0
        while i < len(insts):
            inst = insts[i]
            if inst.__class__.__name__ not in _FIXUP_SKIP:
                si = inst.sync_info
                if si is not None and si.on_wait and len(si.on_wait) > 1:
                    waits = list(si.on_wait)
                    extra, keep = waits[:-1], waits[-1:]
                    inst.sync_info = mybir.SyncInfo(
                        on_wait=keep, on_update=list(si.on_update or []))
                    pool = free_nops.get(inst.engine)
                    for kk, w in enumerate(extra):
                        if not pool:
                            raise RuntimeError(
                                f"out of spare nops for {inst.engine}")
                        nop = pool.pop()
                        nop.sync_info = mybir.SyncInfo(on_wait=[w],
                                                       on_update=[])
                        insts.insert(i + kk, nop)
                    i += len(extra)
            i += 1


_NC_CACHE = None


def _get_nc():
    global _NC_CACHE
    if _NC_CACHE is None:
        _NC_CACHE = build_nc()
    return _NC_CACHE


def _prep_in_maps(inputs):
    x = np.asarray(inputs["x"], dtype=np.float32)
    adj = np.asarray(inputs["adj"], dtype=np.float32)
    W_w = np.asarray(inputs["W_w"], dtype=np.float32)
    W_b = np.asarray(inputs["W_b"], dtype=np.float32)
    A = np.asarray(inputs["A"], dtype=np.float32)
    gate_w = np.asarray(inputs["gate_w"], dtype=np.float32)
    gate_b = np.asarray(inputs["gate_b"], dtype=np.float32)

    cblob = np.zeros((128, CB_W), dtype=np.float32)
    cblob[:, CB_WWT:CB_WWT + 128] = W_w.T
    cblob[:, CB_A:CB_A + 128] = A
    cblob[:, CB_ID:CB_ID + 128] = np.eye(128, dtype=np.float32)
    cblob[:, CB_WB] = W_b
    cblob[:, CB_GW:CB_GW + 2] = gate_w.reshape(2, D).T
    cblob[:, CB_NGB] = -gate_b[0]
    cblob[:, CB_NS] = -S_SHIFT

    in_maps = []
    for c in range(NCORES):
        sl = slice(c * BPC, (c + 1) * BPC)
        adj_c = adj[sl]                                    # [BPC, N, N]
        # adjbias[b, p, jb*N + i] = 0 if adj[b, i, jb*128+p] else -CBIG
        adjT_c = np.ascontiguousarray(
            (adj_c.transpose(0, 2, 1).reshape(BPC, NB, 128, N)
             .transpose(0, 2, 1, 3).reshape(BPC, 128, NB * N) - 1.0)
            * CBIG).astype(ml_dtypes.float8_e4m3fn)
        xnb_c = np.zeros((BPC, 128, N + NB), dtype=np.float32)
        xnb_c[:, :, :N] = x[sl].transpose(0, 2, 1)
        ndeg = (N - adj_c.sum(axis=1)) * np.exp(-S_SHIFT)  # [BPC, N] (j idx)
        xnb_c[:, :, N:] = ndeg.reshape(BPC, NB, 128).transpose(0, 2, 1)
        in_maps.append({"adjT": adjT_c, "xnb": xnb_c, "cblob": cblob})
    return in_maps


def _run(inputs, trace=False, **kwargs):
    nc = _get_nc()
    in_maps = _prep_in_maps(inputs)
    res = run_bass_kernel_spmd(nc, in_maps, core_ids=list(range(NCORES)),
                               trace=trace, **kwargs)
    out = np.concatenate(
        [res.results[c]["out"].reshape(BPC, N, D) for c in range(NCORES)],
        axis=0)
    return out.astype(np.float32), res


def kernel(**inputs) -> np.ndarray:
    out, _ = _run(inputs, trace=False)
    return out
